# revision 11
# baseline (speedup 1.0000x reference)
"""Trainium2 Bass kernel for DeepConvWeigthNet.

Data-parallel across 8 NeuronCores: each core processes one batch image
(B=8). Per core:

  Phase A: fully fused band pipeline over 32 bands of 16 rows.
    conv1 (1->32) -> conv2 (32->64) -> conv3 (64->32) -> heads (32->12)
    All activations bf16, resident in double-buffered SBUF rolling
    replica buffers (2-row tails carried between bands; zero HBM
    roundtrips for bodies). dy-tap replicas built with full-width
    SBUF->SBUF DMA copies (1 descriptor/partition). Bias folded into
    the matmul via a ones-row (conv1 K=10, conv2 K=97) so PReLU can run
    on DVE; conv3/heads PReLU/bias on ACT. Head outputs y staged to
    HBM bf16, 1 DMA per band; CA row sums accumulated on the fly.

  Phase B: CA gating + channel softmax + multiscale box blurs
    (DVE shift-tree along W, banded f32r matmuls along H), weighted
    combines out1 -> out2 -> out3.
"""

import os
import sys

sys.path.insert(0, "/opt/trn_rl_repo")

import numpy as np

H = W = 512
R = 16            # band rows
NB = H // R       # 32 bands
CAP = 22          # replica buffer slots (R + 6)
XW = 520          # padded row width for activation buffers
G = 2             # rows per PSUM group
BS = 560          # phase-B padded block stride
DOFF = 12         # phase-B data col offset within block
NCORES = 8
NPIX = float(H * W)

DEBUG = bool(int(os.environ.get("KBENCH_DEBUG", "0")))
STAGES = os.environ.get("KBENCH_STAGES", "AB")

_CACHE = {}


def _pack_host(inputs):
    """Pack conv weights into the matmul layouts the kernel expects."""
    f = np.float32
    w1 = np.asarray(inputs["w1"], f)   # [32,1,3,3]
    w2 = np.asarray(inputs["w2"], f)   # [64,32,3,3]
    w3 = np.asarray(inputs["w3"], f)   # [32,64,3,3]
    hws = [np.asarray(inputs[f"hw{i}"], f) for i in (1, 2, 3)]  # [4,32,3,3]

    # conv1: lhsT [10, 32], row 0 = bias, row 1+3a+b -> w1[co,0,a,b]
    w1m = np.zeros((10, 32), f)
    for a in range(3):
        for b in range(3):
            w1m[1 + 3 * a + b, :] = w1[:, 0, a, b]
    w1m[0, :] = np.asarray(inputs["b1"], f)

    # conv2: per dx b: lhsT [97, 64], row 32a+ci; row 96 = bias iff b==1
    w2m = np.zeros((3, 97, 64), f)
    for b in range(3):
        for a in range(3):
            w2m[b, 32 * a:32 * a + 32, :] = w2[:, :, a, b].T
    w2m[1, 96, :] = np.asarray(inputs["b2"], f)

    # conv3: mm1 K=128 covers a=0,1 ; mm2 K=64 covers a=2 (bias via ACT)
    w3a = np.zeros((3, 128, 32), f)
    w3b = np.zeros((3, 64, 32), f)
    for b in range(3):
        for a in range(2):
            w3a[b, 64 * a:64 * a + 64, :] = w3[:, :, a, b].T
        w3b[b, :, :] = w3[:, :, 2, b].T

    # heads fused: lhsT [96, 12], col 4h+co (bias via ACT)
    hwm = np.zeros((3, 96, 12), f)
    for b in range(3):
        for a in range(3):
            for hI, hw in enumerate(hws):
                hwm[b, 32 * a:32 * a + 32, 4 * hI:4 * hI + 4] = hw[:, :, a, b].T

    # CA 1x1 convs as block-diagonal [12,12] lhsT (row = in ch, col = out)
    def blockdiag(ws):
        m = np.zeros((12, 12), f)
        for i, wca in enumerate(ws):
            m[4 * i:4 * i + 4, 4 * i:4 * i + 4] = wca[:, :, 0, 0].T
        return m

    caA = blockdiag([np.asarray(inputs[f"ca{i}a"], f) for i in (1, 2, 3)]) / NPIX
    caB = blockdiag([np.asarray(inputs[f"ca{i}b"], f) for i in (1, 2, 3)])

    # banded along-H blur matrices: [kidx, t, rel] -> [128 in, 128 out]
    ks = (5, 15, 25)
    bandH = np.zeros((3, 4, 3, 128, 128), f)
    for kidx, k in enumerate(ks):
        c = (k - 1) // 2
        inv = 1.0 / (k * k)
        for t in range(4):
            for relidx, rel in enumerate((-1, 0, 1)):
                tp = t + rel
                if tp < 0 or tp > 3:
                    continue
                ii = np.arange(128)[:, None] + 128 * tp
                jj = np.arange(128)[None, :] + 128 * t
                bandH[kidx, t, relidx][np.abs(ii - jj) <= c] = inv
    bandP = np.ascontiguousarray(
        np.transpose(bandH, (3, 0, 1, 2, 4)).reshape(128, 36 * 128))

    biases = {
        "b3": np.asarray(inputs["b3"], f).reshape(32, 1),
        "hb": np.concatenate([np.asarray(inputs[f"hb{i}"], f)
                              for i in (1, 2, 3)]).reshape(12, 1),
    }
    return dict(w1m=w1m, w2m=w2m, w3a=w3a, w3b=w3b, hwm=hwm,
                caA=caA, caB=caB, bandP=bandP, **biases)


def _build(alpha1, alpha2, alpha3, debug=False, loop_reps=0, stages="AB"):
    import concourse.bacc as bacc
    import concourse.mybir as mybir
    import concourse.tile as tile
    from concourse import bass

    dt = mybir.dt
    AFT = mybir.ActivationFunctionType

    nc = bacc.Bacc("TRN2", target_bir_lowering=False, debug=False,
                   num_devices=NCORES)

    # ---- I/O ----
    xb = nc.dram_tensor("xb", [H, W], dt.float32, kind="ExternalInput")
    w1m_d = nc.dram_tensor("w1m", [10, 32], dt.float32, kind="ExternalInput")
    w2m_d = nc.dram_tensor("w2m", [3, 97, 64], dt.float32, kind="ExternalInput")
    w3a_d = nc.dram_tensor("w3a", [3, 128, 32], dt.float32, kind="ExternalInput")
    w3b_d = nc.dram_tensor("w3b", [3, 64, 32], dt.float32, kind="ExternalInput")
    hwm_d = nc.dram_tensor("hwm", [3, 96, 12], dt.float32, kind="ExternalInput")
    caA_d = nc.dram_tensor("caA", [12, 12], dt.float32, kind="ExternalInput")
    caB_d = nc.dram_tensor("caB", [12, 12], dt.float32, kind="ExternalInput")
    bandP_d = nc.dram_tensor("bandP", [128, 36 * 128], dt.float32,
                             kind="ExternalInput")
    b3_d = nc.dram_tensor("b3", [32, 1], dt.float32, kind="ExternalInput")
    hb_d = nc.dram_tensor("hb", [12, 1], dt.float32, kind="ExternalInput")

    outb = nc.dram_tensor("outb", [H, W], dt.float32, kind="ExternalOutput")

    dbg = {}
    if debug:
        dbg["d_y"] = nc.dram_tensor("d_y", [12, H, W], dt.bfloat16,
                                    kind="ExternalOutput")
        dbg["d_g"] = nc.dram_tensor("d_g", [12, 1], dt.float32,
                                    kind="ExternalOutput")

    with tile.TileContext(nc) as tc:
        with (
            tc.tile_pool(name="dram", bufs=1, space="DRAM") as dpool,
            tc.tile_pool(name="wsb", bufs=1) as wsb,
        ):
            x_row = dpool.tile([H + 2, XW], dt.bfloat16)   # padded bf16 rows
            y_dram = dpool.tile([12, H, W], dt.bfloat16)

            # packed weights: one bf16 tile, one f32 tile
            # wcat cols: w2 [0:192) on parts 0:97; w3a [192:288) parts 0:128;
            #            w3b [288:384) parts 0:64; hw [384:420) parts 0:96;
            #            w1 [420:452) parts 0:10
            wcat = wsb.tile([128, 452], dt.bfloat16)
            w2sb = wcat[0:97, 0:192]
            w3asb = wcat[0:128, 192:288]
            w3bsb = wcat[64:128, 288:384]
            hwsb = wcat[0:96, 384:420]
            w1sb = wcat[0:10, 420:452]
            # fcat cols: caA [0:12), caB [12:24), b3 [24:25), hb [25:26),
            #            ones row [26:27) col-major? ones needs [1,128]:
            #            use fcat[0:1, 32:160]; accums [12, 160:416)
            fcat = wsb.tile([128, 416], dt.float32)
            caAsb = fcat[0:12, 0:12]
            caBsb = fcat[0:12, 12:24]
            b3sb = fcat[0:32, 24:25]
            hbsb = fcat[0:12, 25:26]
            onesb = fcat[0:1, 32:160]
            accums = fcat[0:12, 160:160 + NB * (R // G)]

            with tc.tile_pool(name="wstg", bufs=1) as wstg:
                wf = wstg.tile([128, 452], dt.float32)
                nc.vector.memset(wf[:], 0.0)
                for b in range(3):
                    nc.sync.dma_start(wf[0:97, b * 64:(b + 1) * 64], w2m_d[b])
                    nc.sync.dma_start(wf[0:128, 192 + b * 32:192 + (b + 1) * 32],
                                      w3a_d[b])
                    nc.sync.dma_start(wf[64:128, 288 + b * 32:288 + (b + 1) * 32],
                                      w3b_d[b])
                    nc.sync.dma_start(wf[0:96, 384 + b * 12:384 + (b + 1) * 12],
                                      hwm_d[b])
                nc.sync.dma_start(wf[0:10, 420:452], w1m_d[:])
                nc.vector.tensor_copy(wcat[:], wf[:])

            nc.sync.dma_start(caAsb, caA_d[:])
            nc.sync.dma_start(caBsb, caB_d[:])
            nc.sync.dma_start(b3sb, b3_d[:])
            nc.sync.dma_start(hbsb, hb_d[:])
            nc.vector.memset(onesb, 1.0)

            # ---- stage A0: x -> bf16 padded-row DRAM tensor ----
            def stage_a0():
                with tc.tile_pool(name="a0", bufs=1) as a0:
                    xt = a0.tile([128, 4, 512], dt.float32, name="xt")
                    xsb = a0.tile([128, 4, XW], dt.bfloat16, name="xsb")
                    zz = a0.tile([2, XW], dt.bfloat16, name="zz")
                    nc.sync.dma_start(
                        xt[:], xb[:, :].rearrange("(b p) w -> p b w", p=128))
                    nc.vector.memset(xsb[:], 0.0)
                    nc.vector.memset(zz[:], 0.0)
                    nc.vector.tensor_copy(xsb[:, :, 1:513], xt[:])
                    nc.sync.dma_start(
                        x_row[0:H, :].rearrange("(b p) w -> p b w", p=128),
                        xsb[:])
                    nc.sync.dma_start(x_row[H:H + 2, :], zz[:])

            # ============ phase A: fused band pipeline ============
            def stage_a():
                with (
                    tc.tile_pool(name="abuf", bufs=1) as abuf,
                    tc.tile_pool(name="ymid", bufs=1) as ymid,
                    tc.tile_pool(name="atmp", bufs=1) as atmp,
                    tc.tile_pool(name="aps", bufs=4, space="PSUM") as aps,
                ):
                    xrep = abuf.tile([10, 19, XW], dt.bfloat16, name="xrep")
                    b1r = [abuf.tile([97, CAP, XW], dt.bfloat16,
                                     name=f"b1r{i}") for i in range(2)]
                    b2r = [abuf.tile([128, CAP, XW], dt.bfloat16,
                                     name=f"b2r{i}") for i in range(2)]
                    b3r = [abuf.tile([96, CAP, XW], dt.bfloat16,
                                     name=f"b3r{i}") for i in range(2)]
                    # zero-init: unwritten slots read as zero padding
                    for t in b1r + b2r + b3r:
                        nc.vector.memset(t[:], 0.0)
                    nc.vector.memset(xrep[:], 0.0)
                    nc.vector.memset(xrep[0:1, :, :], 1.0)
                    for t in b1r:
                        nc.vector.memset(t[96:97, :, :], 1.0)

                    xr_handle = x_row[:, :]

                    def xgather(k, lo1, hi1):
                        """xrep[1+3a+b, u, c] = x_row[lo1-1+u+a, c+b]"""
                        n1 = hi1 - lo1
                        for a in range(3):
                            s0 = max(lo1 - 1 + a, 0)   # rows >= H are zero pad
                            s1 = hi1 - 1 + a
                            u0 = s0 - (lo1 - 1 + a)
                            cnt = s1 - s0
                            if cnt <= 0:
                                continue
                            src = bass.AP(
                                xr_handle.tensor,
                                xr_handle.offset + s0 * XW,
                                [[1, 3], [XW, cnt], [1, 514]])
                            nc.sync.dma_start(
                                xrep[1 + 3 * a:4 + 3 * a, u0:u0 + cnt, 0:514],
                                src)

                    def prelu_dve(dst, ps, alpha, g, p):
                        tmp = atmp.tile([64, G, 512], dt.float32, tag="ptmp",
                                        name="ptmp")
                        nc.vector.tensor_scalar_mul(
                            tmp[0:p, 0:g, :], ps[0:p, 0:g, :], alpha)
                        nc.vector.tensor_max(dst, tmp[0:p, 0:g, :],
                                             ps[0:p, 0:g, :])

                    def conv1(k, buf):
                        r0 = R * k
                        lo1 = r0 + 3 if k else 0
                        hi1 = min(r0 + R + 3, H)
                        xgather(k, lo1, hi1)
                        q = lo1
                        while q < hi1:
                            g = min(G, hi1 - q)
                            ps = aps.tile([64, G, 512], dt.float32, tag="ps",
                                          name="psT")
                            for j in range(g):
                                u = (q + j) - lo1
                                nc.tensor.matmul(ps[0:32, j, :], w1sb,
                                                 xrep[0:10, u, 0:512],
                                                 start=True, stop=True)
                            s0 = (q - r0) + 3
                            dst = buf[0:32, s0:s0 + g, 1:513]
                            prelu_dve(dst, ps, alpha1, g, 32)
                            q += g
                        s_hi = (hi1 - r0) + 3
                        if s_hi < CAP:
                            nc.vector.memset(buf[0:32, s_hi:CAP, :], 0.0)
                        # replicas: g1 <- g0(+1), g2 <- g0(+2)
                        c1 = (lo1 - r0) + 2
                        nc.sync.dma_start(buf[32:64, c1:21, :],
                                          buf[0:32, c1 + 1:22, :])
                        nc.sync.dma_start(buf[64:96, c1 - 1:20, :],
                                          buf[0:32, c1 + 1:22, :])

                    def conv2(k, src, buf):
                        r0 = R * k
                        lo2 = r0 + 2 if k else 0
                        hi2 = min(r0 + R + 2, H)
                        q = lo2
                        while q < hi2:
                            g = min(G, hi2 - q)
                            ps = aps.tile([64, G, 512], dt.float32, tag="ps",
                                          name="psT")
                            for j in range(g):
                                t1 = (q + j) - r0 + 2
                                for b in range(3):
                                    nc.tensor.matmul(
                                        ps[:, j, :],
                                        w2sb[:, b * 64:(b + 1) * 64],
                                        src[0:97, t1, b:b + 512],
                                        start=(b == 0), stop=(b == 2))
                            s0 = (q - r0) + 4
                            dst = buf[0:64, s0:s0 + g, 1:513]
                            prelu_dve(dst, ps, alpha2, g, 64)
                            q += g
                        s_hi = (hi2 - r0) + 4
                        if s_hi < CAP:
                            nc.vector.memset(buf[0:64, s_hi:CAP, :], 0.0)
                        c2 = (lo2 - r0) + 3
                        nc.sync.dma_start(buf[64:128, c2:21, :],
                                          buf[0:64, c2 + 1:22, :])

                    def conv3(k, src, buf):
                        r0 = R * k
                        lo3 = r0 + 1 if k else 0
                        hi3 = min(r0 + R + 1, H)
                        q = lo3
                        while q < hi3:
                            g = min(G, hi3 - q)
                            ps = aps.tile([64, G, 512], dt.float32, tag="ps",
                                          name="psT")
                            for j in range(g):
                                t2 = (q + j) - r0 + 3
                                for b in range(3):
                                    nc.tensor.matmul(
                                        ps[0:32, j, :],
                                        w3asb[:, b * 32:(b + 1) * 32],
                                        src[0:128, t2, b:b + 512],
                                        start=(b == 0), stop=False)
                                for b in range(3):
                                    nc.tensor.matmul(
                                        ps[0:32, j, :],
                                        w3bsb[:, b * 32:(b + 1) * 32],
                                        src[64:128, t2 + 1, b:b + 512],
                                        start=False, stop=(b == 2))
                            s0 = (q - r0) + 5
                            nc.scalar.activation(buf[0:32, s0:s0 + g, 1:513],
                                                 ps[0:32, 0:g, :], AFT.Prelu,
                                                 bias=b3sb, scale=1.0,
                                                 alpha=alpha3)
                            q += g
                        s_hi = (hi3 - r0) + 5
                        if s_hi < CAP:
                            nc.vector.memset(buf[0:32, s_hi:CAP, :], 0.0)
                        c3 = (lo3 - r0) + 4
                        nc.sync.dma_start(buf[32:64, c3:21, :],
                                          buf[0:32, c3 + 1:22, :])
                        nc.sync.dma_start(buf[64:96, c3 - 1:20, :],
                                          buf[0:32, c3 + 1:22, :])

                    def heads(k, src):
                        r0 = R * k
                        ystg = ymid.tile([12, R, 512], dt.bfloat16,
                                         tag="ystg", name="ystg")
                        for jj in range(R // G):
                            ps = aps.tile([64, G, 512], dt.float32, tag="ps",
                                          name="psT")
                            for j in range(G):
                                th = jj * G + j + 4
                                for b in range(3):
                                    nc.tensor.matmul(
                                        ps[0:12, j, :],
                                        hwsb[:, b * 12:(b + 1) * 12],
                                        src[0:96, th, b:b + 512],
                                        start=(b == 0), stop=(b == 2))
                            idx = k * (R // G) + jj
                            nc.scalar.activation(
                                ystg[:, jj * G:(jj + 1) * G, :],
                                ps[0:12, :, :], AFT.Identity, bias=hbsb,
                                scale=1.0, accum_out=accums[:, idx:idx + 1])
                        nc.sync.dma_start(y_dram[:, r0:r0 + R, :], ystg[:])

                    for k in range(NB):
                        cur1, cur2, cur3 = b1r[k % 2], b2r[k % 2], b3r[k % 2]
                        if k > 0:
                            prv1, prv2, prv3 = (b1r[1 - k % 2], b2r[1 - k % 2],
                                                b3r[1 - k % 2])
                            nc.sync.dma_start(cur1[0:96, 4:6, :],
                                              prv1[0:96, 20:22, :])
                            nc.sync.dma_start(cur2[0:128, 4:6, :],
                                              prv2[0:128, 20:22, :])
                            nc.sync.dma_start(cur3[0:96, 4:6, :],
                                              prv3[0:96, 20:22, :])
                        conv1(k, cur1)
                        conv2(k, cur1, cur2)
                        conv3(k, cur2, cur3)
                        heads(k, cur3)

            # ============ phase B: CA + softmax + blurs ============
            def stage_b():
                with (
                    tc.tile_pool(name="bsm", bufs=1) as bsm,
                    tc.tile_pool(name="bps1", bufs=1, space="PSUM") as bps1,
                    tc.tile_pool(name="bps", bufs=2, space="PSUM") as bps,
                    tc.tile_pool(name="bbl", bufs=1) as bbl,
                ):
                    bandf = bsm.tile([128, 36 * 128], dt.float32, name="bandf")
                    bandsb = bsm.tile([128, 36 * 128], dt.float32r,
                                      name="bandsb")
                    nc.sync.dma_start(bandf[:], bandP_d[:])
                    nc.vector.tensor_copy(bandsb[:], bandf[:])

                    # CA gating
                    total = bsm.tile([12, 1], dt.float32, name="total")
                    nc.vector.reduce_sum(total[:], accums,
                                         axis=mybir.AxisListType.X)
                    psA = bps1.tile([12, 1], dt.float32, tag="caps", name="psA")
                    nc.tensor.matmul(psA[:], caAsb, total[:],
                                     start=True, stop=True)
                    trelu = bsm.tile([12, 1], dt.float32, name="trelu")
                    nc.scalar.activation(trelu[:], psA[:], AFT.Relu)
                    psB = bps1.tile([12, 1], dt.float32, tag="caps", name="psB")
                    nc.tensor.matmul(psB[:], caBsb, trelu[:],
                                     start=True, stop=True)
                    g_gate = bsm.tile([12, 1], dt.float32, name="g_gate")
                    nc.scalar.activation(g_gate[:], psB[:], AFT.Sigmoid)
                    if debug:
                        nc.sync.dma_start(dbg["d_g"][:], g_gate[:])
                        nc.sync.dma_start(dbg["d_y"][:], y_dram[:])
                    g_row = bsm.tile([1, 12], dt.float32, name="g_row")
                    nc.sync.dma_start(g_row[:], g_gate[:])
                    psG = bps1.tile([128, 12], dt.float32, tag="gbc", name="psG")
                    nc.tensor.matmul(psG[:], onesb, g_row[:],
                                     start=True, stop=True)
                    gbc = bsm.tile([128, 12], dt.float32, name="gbc")
                    nc.vector.tensor_copy(gbc[:], psG[:])

                    # blur planes
                    FW = 4 * BS  # 2240
                    u = bbl.tile([128, FW], dt.float32r, name="u")
                    S2 = bbl.tile([128, FW], dt.float32r, name="S2")
                    S4 = bbl.tile([128, FW], dt.float32r, name="S4")
                    S8 = bbl.tile([128, FW], dt.float32r, name="S8")
                    S16 = bbl.tile([128, FW], dt.float32r, name="S16")
                    S5 = bbl.tile([128, FW], dt.float32r, name="S5")
                    S15 = bbl.tile([128, FW], dt.float32r, name="S15")
                    S25 = bbl.tile([128, FW], dt.float32r, name="S25")
                    unext = bbl.tile([128, FW], dt.float32r, name="unext")
                    t1 = bbl.tile([128, 512], dt.float32, name="t1")
                    t2 = bbl.tile([128, 512], dt.float32, name="t2")
                    ostg = bbl.tile([128, 4, 512], dt.float32, name="ostg")
                    nc.vector.memset(u[:].bitcast(dt.float32), 0.0)
                    nc.vector.memset(unext[:].bitcast(dt.float32), 0.0)

                    xt2 = bsm.tile([128, 4, 512], dt.float32, name="xt2")
                    nc.sync.dma_start(
                        xt2[:], xb[:, :].rearrange("(b p) w -> p b w", p=128))
                    uview = u[:].rearrange("p (b w) -> p b w", b=4)
                    nc.vector.tensor_copy(uview[:, :, DOFF:DOFF + 512], xt2[:])

                    yt = bsm.tile([128, 4, 4, 512], dt.bfloat16, name="yt")
                    ep = [bsm.tile([128, 4, 512], dt.float32, tag=f"exp{c}",
                                   name=f"ep{c}")
                          for c in range(4)]
                    tsum = bsm.tile([128, 4, 512], dt.float32, name="tsum")

                    cs = {5: 2, 15: 7, 25: 12}
                    ucur, unxt = u, unext
                    for stage in range(3):
                        # softmax for this head (channels 4*stage .. +4)
                        nc.sync.dma_start(
                            yt[:],
                            y_dram[4 * stage:4 * stage + 4, :, :].rearrange(
                                "c (b p) w -> p c b w", p=128))
                        for c in range(4):
                            cg = 4 * stage + c
                            nc.scalar.activation(ep[c][:], yt[:, c, :, :],
                                                 AFT.Exp,
                                                 scale=gbc[:, cg:cg + 1])
                        nc.vector.tensor_add(tsum[:], ep[0][:], ep[1][:])
                        nc.vector.tensor_add(tsum[:], tsum[:], ep[2][:])
                        nc.vector.tensor_add(tsum[:], tsum[:], ep[3][:])
                        nc.vector.reciprocal(tsum[:], tsum[:])
                        for c in range(4):
                            nc.vector.tensor_mul(ep[c][:], ep[c][:], tsum[:])

                        # shift-tree along W (horizontal box sums)
                        wv = FW - 24
                        nc.vector.tensor_add(S2[:, 0:wv], ucur[:, 0:wv],
                                             ucur[:, 1:1 + wv])
                        nc.vector.tensor_add(S4[:, 0:wv], S2[:, 0:wv],
                                             S2[:, 2:2 + wv])
                        nc.vector.tensor_add(S8[:, 0:wv], S4[:, 0:wv],
                                             S4[:, 4:4 + wv])
                        nc.vector.tensor_add(S16[:, 0:wv], S8[:, 0:wv],
                                             S8[:, 8:8 + wv])
                        nc.vector.tensor_add(S5[:, 0:wv], S4[:, 0:wv],
                                             ucur[:, 4:4 + wv])
                        nc.vector.tensor_sub(S15[:, 0:wv], S16[:, 0:wv],
                                             ucur[:, 15:15 + wv])
                        nc.vector.tensor_add(S25[:, 0:wv], S16[:, 0:wv],
                                             S8[:, 16:16 + wv])
                        nc.vector.tensor_add(S25[:, 0:wv], S25[:, 0:wv],
                                             ucur[:, 24:24 + wv])

                        Sk = {5: S5, 15: S15, 25: S25}
                        for t in range(4):
                            pk = {}
                            for kidx, kk in enumerate((5, 15, 25)):
                                ps = bps.tile([128, 512], dt.float32,
                                              tag=f"blur{kidx}",
                                              name=f"blur{kidx}")
                                rels = [r for r in (-1, 0, 1)
                                        if 0 <= t + r <= 3]
                                for ri, rel in enumerate(rels):
                                    idx = kidx * 12 + t * 3 + (rel + 1)
                                    off = (t + rel) * BS + DOFF - cs[kk]
                                    nc.tensor.matmul(
                                        ps[:],
                                        bandsb[:, idx * 128:(idx + 1) * 128],
                                        Sk[kk][:, off:off + 512],
                                        start=(ri == 0),
                                        stop=(ri == len(rels) - 1))
                                pk[kk] = ps
                            ub = ucur[:, t * BS + DOFF:t * BS + DOFF + 512]
                            nc.vector.tensor_mul(t1[:], ep[0][:, t, :], ub)
                            nc.vector.tensor_mul(t2[:], ep[1][:, t, :],
                                                 pk[5][:])
                            nc.vector.tensor_add(t1[:], t1[:], t2[:])
                            nc.vector.tensor_mul(t2[:], ep[2][:, t, :],
                                                 pk[15][:])
                            nc.vector.tensor_add(t1[:], t1[:], t2[:])
                            nc.vector.tensor_mul(t2[:], ep[3][:, t, :],
                                                 pk[25][:])
                            if stage < 2:
                                nc.vector.tensor_add(
                                    unxt[:, t * BS + DOFF:t * BS + DOFF + 512],
                                    t1[:], t2[:])
                            else:
                                nc.vector.tensor_add(ostg[:, t, :], t1[:],
                                                     t2[:])
                        if stage < 2:
                            ucur, unxt = unxt, ucur

                    nc.sync.dma_start(
                        outb[:, :].rearrange("(b p) w -> p b w", p=128),
                        ostg[:])

            def phases():
                if "A" in stages:
                    stage_a0()
                    stage_a()
                if "B" in stages:
                    stage_b()

            if loop_reps:
                with tc.For_i(0, loop_reps, 1):
                    phases()
            else:
                phases()

    nc.compile()
    return nc


class _Runner:
    """Cached PJRT runner: jit/NEFF compile once, execute many times."""

    def __init__(self, nc):
        import jax
        import concourse.mybir as mybir
        from concourse import bass2jax
        from jax.sharding import Mesh, PartitionSpec
        from jax.experimental.shard_map import shard_map

        bass2jax.install_neuronx_cc_hook()
        self.nc = nc
        in_names, out_names, out_avals, zero_outs = [], [], [], []
        partition_name = (nc.partition_id_tensor.name
                          if nc.partition_id_tensor else None)
        for alloc in nc.m.functions[0].allocations:
            if not isinstance(alloc, mybir.MemoryLocationSet):
                continue
            name = alloc.memorylocations[0].name
            if alloc.kind == "ExternalInput":
                if name != partition_name:
                    in_names.append(name)
            elif alloc.kind == "ExternalOutput":
                out_names.append(name)
                shape = tuple(alloc.tensor_shape)
                dtype = mybir.dt.np(alloc.dtype)
                out_avals.append(jax.core.ShapedArray(shape, dtype))
                zero_outs.append(np.zeros(shape, dtype))
        self.in_names = list(in_names)
        self.out_names = out_names
        self.out_avals = out_avals
        self.zero_outs = zero_outs
        n_params = len(in_names)
        n_outs = len(out_names)
        all_names = in_names + out_names
        if partition_name is not None:
            all_names.append(partition_name)

        def _body(*args):
            operands = list(args)
            if partition_name is not None:
                operands.append(bass2jax.partition_id_tensor())
            outs = bass2jax._bass_exec_p.bind(
                *operands,
                out_avals=tuple(out_avals),
                in_names=tuple(all_names),
                out_names=tuple(out_names),
                lowering_input_output_aliases=(),
                sim_require_finite=True,
                sim_require_nnan=True,
                nc=nc,
            )
            return tuple(outs)

        devices = jax.devices()[:NCORES]
        mesh = Mesh(np.asarray(devices), ("core",))
        in_specs = (PartitionSpec("core"),) * (n_params + n_outs)
        out_specs = (PartitionSpec("core"),) * n_outs
        self.sharded = jax.jit(
            shard_map(_body, mesh=mesh, in_specs=in_specs, out_specs=out_specs,
                      check_rep=False),
            keep_unused=True,
        )

    def concat_inputs(self, in_maps):
        return [
            np.concatenate([np.asarray(in_maps[c][nm]) for c in range(NCORES)],
                           axis=0)
            for nm in self.in_names
        ]

    def concat_zeros(self):
        return [np.zeros((NCORES * z.shape[0], *z.shape[1:]), z.dtype)
                for z in self.zero_outs]

    def __call__(self, in_maps):
        out_arrs = self.sharded(*self.concat_inputs(in_maps),
                                *self.concat_zeros())
        return [
            {nm: np.asarray(out_arrs[i]).reshape(NCORES,
                                                 *self.out_avals[i].shape)[c]
             for i, nm in enumerate(self.out_names)}
            for c in range(NCORES)
        ]


def _get_runner(alpha1, alpha2, alpha3, loop_reps=0, stages=None):
    if stages is None:
        stages = STAGES
    key = ("runner", alpha1, alpha2, alpha3, DEBUG, loop_reps, stages)
    if key not in _CACHE:
        key_nc = (alpha1, alpha2, alpha3, DEBUG, loop_reps, stages)
        if key_nc not in _CACHE:
            _CACHE[key_nc] = _build(alpha1, alpha2, alpha3, debug=DEBUG,
                                    loop_reps=loop_reps, stages=stages)
        _CACHE[key] = _Runner(_CACHE[key_nc])
    return _CACHE[key]


def make_in_maps(inputs):
    x = np.asarray(inputs["x"], np.float32)   # [8,1,512,512]
    packed = _pack_host(inputs)
    in_maps = []
    for i in range(NCORES):
        m = {"xb": np.ascontiguousarray(x[i, 0])}
        m.update({k: packed[k] for k in ("w1m", "w2m", "w3a", "w3b", "hwm",
                                         "caA", "caB", "bandP", "b3", "hb")})
        in_maps.append(m)
    return in_maps


def kernel(**inputs):
    runner = _get_runner(float(inputs["a1"]), float(inputs["a2"]),
                         float(inputs["a3"]))
    results = runner(make_in_maps(inputs))
    out = np.stack([results[i]["outb"] for i in range(NCORES)])
    globals()["_LAST_RESULTS"] = results
    return out.reshape(8, 1, H, W).astype(np.float32)


# revision 20
# speedup vs baseline: 2.3648x; 2.3648x over previous
"""Trainium2 Bass kernel for DeepConvWeigthNet.

Data-parallel across 8 NeuronCores: each core processes one batch image
(B=8). Per core:

  Phase A: fully fused band pipeline over 32 bands of 16 rows.
    conv1 (1->32) -> conv2 (32->64) -> conv3 (64->32) -> heads (32->12)
    All activations bf16, resident in double-buffered SBUF rolling
    replica buffers (2-row tails carried between bands; zero HBM
    roundtrips for bodies). dy-tap replicas built with full-width
    SBUF->SBUF DMA copies (1 descriptor/partition). Bias folded into
    the matmul via a ones-row (conv1 K=10, conv2 K=97) so PReLU can run
    on DVE; conv3/heads PReLU/bias on ACT. Head outputs y staged to
    HBM bf16, 1 DMA per band; CA row sums accumulated on the fly.

  Phase B: CA gating + channel softmax + multiscale box blurs
    (DVE shift-tree along W, banded f32r matmuls along H), weighted
    combines out1 -> out2 -> out3.
"""

import os
import sys

sys.path.insert(0, "/opt/trn_rl_repo")

import numpy as np

H = W = 512
R = 16            # band rows
NB = H // R       # 32 bands
CAP = 22          # replica buffer slots (R + 6)
XW = 520          # padded row width for activation buffers
G = 2             # rows per PSUM group
BS = 560          # phase-B padded block stride
DOFF = 12         # phase-B data col offset within block
NCORES = 8
NPIX = float(H * W)

DEBUG = bool(int(os.environ.get("KBENCH_DEBUG", "0")))
STAGES = os.environ.get("KBENCH_STAGES", "AB")
SKIP = os.environ.get("KBENCH_SKIP", "")   # timing knobs: c,g,p,m

_CACHE = {}


def _pack_host(inputs):
    """Pack conv weights into the matmul layouts the kernel expects."""
    f = np.float32
    w1 = np.asarray(inputs["w1"], f)   # [32,1,3,3]
    w2 = np.asarray(inputs["w2"], f)   # [64,32,3,3]
    w3 = np.asarray(inputs["w3"], f)   # [32,64,3,3]
    hws = [np.asarray(inputs[f"hw{i}"], f) for i in (1, 2, 3)]  # [4,32,3,3]

    # conv1: lhsT [10, 32], row 0 = bias, row 1+3a+b -> w1[co,0,a,b]
    w1m = np.zeros((10, 32), f)
    for a in range(3):
        for b in range(3):
            w1m[1 + 3 * a + b, :] = w1[:, 0, a, b]
    w1m[0, :] = np.asarray(inputs["b1"], f)

    # conv2: per dx b: lhsT [97, 64], row 32a+ci; row 96 = bias iff b==1
    w2m = np.zeros((3, 97, 64), f)
    for b in range(3):
        for a in range(3):
            w2m[b, 32 * a:32 * a + 32, :] = w2[:, :, a, b].T
    w2m[1, 96, :] = np.asarray(inputs["b2"], f)

    # conv3: mm1 K=128 covers a=0,1 ; mm2 K=64 covers a=2 (bias via ACT)
    w3a = np.zeros((3, 128, 32), f)
    w3b = np.zeros((3, 64, 32), f)
    for b in range(3):
        for a in range(2):
            w3a[b, 64 * a:64 * a + 64, :] = w3[:, :, a, b].T
        w3b[b, :, :] = w3[:, :, 2, b].T

    # heads fused: lhsT [96, 12], col 4h+co (bias via ACT)
    hwm = np.zeros((3, 96, 12), f)
    for b in range(3):
        for a in range(3):
            for hI, hw in enumerate(hws):
                hwm[b, 32 * a:32 * a + 32, 4 * hI:4 * hI + 4] = hw[:, :, a, b].T

    # CA 1x1 convs as block-diagonal [12,12] lhsT (row = in ch, col = out)
    def blockdiag(ws):
        m = np.zeros((12, 12), f)
        for i, wca in enumerate(ws):
            m[4 * i:4 * i + 4, 4 * i:4 * i + 4] = wca[:, :, 0, 0].T
        return m

    caA = blockdiag([np.asarray(inputs[f"ca{i}a"], f) for i in (1, 2, 3)]) / NPIX
    caB = blockdiag([np.asarray(inputs[f"ca{i}b"], f) for i in (1, 2, 3)])

    # banded along-H blur matrices: [kidx, t, rel] -> [128 in, 128 out]
    ks = (5, 15, 25)
    bandH = np.zeros((3, 4, 3, 128, 128), f)
    for kidx, k in enumerate(ks):
        c = (k - 1) // 2
        inv = 1.0 / (k * k)
        for t in range(4):
            for relidx, rel in enumerate((-1, 0, 1)):
                tp = t + rel
                if tp < 0 or tp > 3:
                    continue
                ii = np.arange(128)[:, None] + 128 * tp
                jj = np.arange(128)[None, :] + 128 * t
                bandH[kidx, t, relidx][np.abs(ii - jj) <= c] = inv
    bandP = np.ascontiguousarray(
        np.transpose(bandH, (3, 0, 1, 2, 4)).reshape(128, 36 * 128))

    biases = {
        "b3": np.asarray(inputs["b3"], f).reshape(32, 1),
        "hb": np.concatenate([np.asarray(inputs[f"hb{i}"], f)
                              for i in (1, 2, 3)]).reshape(12, 1),
    }
    return dict(w1m=w1m, w2m=w2m, w3a=w3a, w3b=w3b, hwm=hwm,
                caA=caA, caB=caB, bandP=bandP, **biases)


def _build(alpha1, alpha2, alpha3, debug=False, loop_reps=0, stages="AB",
           skip=None):
    if skip is None:
        skip = SKIP
    import concourse.bacc as bacc
    import concourse.mybir as mybir
    import concourse.tile as tile
    from concourse import bass

    dt = mybir.dt
    AFT = mybir.ActivationFunctionType

    nc = bacc.Bacc("TRN2", target_bir_lowering=False, debug=False,
                   num_devices=NCORES)

    # ---- I/O ----
    xb = nc.dram_tensor("xb", [H, W], dt.float32, kind="ExternalInput")
    w1m_d = nc.dram_tensor("w1m", [10, 32], dt.float32, kind="ExternalInput")
    w2m_d = nc.dram_tensor("w2m", [3, 97, 64], dt.float32, kind="ExternalInput")
    w3a_d = nc.dram_tensor("w3a", [3, 128, 32], dt.float32, kind="ExternalInput")
    w3b_d = nc.dram_tensor("w3b", [3, 64, 32], dt.float32, kind="ExternalInput")
    hwm_d = nc.dram_tensor("hwm", [3, 96, 12], dt.float32, kind="ExternalInput")
    caA_d = nc.dram_tensor("caA", [12, 12], dt.float32, kind="ExternalInput")
    caB_d = nc.dram_tensor("caB", [12, 12], dt.float32, kind="ExternalInput")
    bandP_d = nc.dram_tensor("bandP", [128, 36 * 128], dt.float32,
                             kind="ExternalInput")
    b3_d = nc.dram_tensor("b3", [32, 1], dt.float32, kind="ExternalInput")
    hb_d = nc.dram_tensor("hb", [12, 1], dt.float32, kind="ExternalInput")

    outb = nc.dram_tensor("outb", [H, W], dt.float32, kind="ExternalOutput")

    dbg = {}
    if debug:
        dbg["d_y"] = nc.dram_tensor("d_y", [12, H, W], dt.bfloat16,
                                    kind="ExternalOutput")
        dbg["d_g"] = nc.dram_tensor("d_g", [12, 1], dt.float32,
                                    kind="ExternalOutput")

    with tile.TileContext(nc) as tc:
        with (
            tc.tile_pool(name="dram", bufs=1, space="DRAM") as dpool,
            tc.tile_pool(name="wsb", bufs=1) as wsb,
        ):
            x_row = dpool.tile([H + 2, XW], dt.bfloat16)   # padded bf16 rows
            y_dram = dpool.tile([12, H, W], dt.bfloat16)

            # packed weights: one bf16 tile, one f32 tile
            # wcat cols: w2 [0:192) on parts 0:97; w3a [192:288) parts 0:128;
            #            w3b [288:384) parts 0:64; hw [384:420) parts 0:96;
            #            w1 [420:452) parts 0:10
            wcat = wsb.tile([128, 452], dt.bfloat16)
            w2sb = wcat[0:97, 0:192]
            w3asb = wcat[0:128, 192:288]
            w3bsb = wcat[64:128, 288:384]
            hwsb = wcat[0:96, 384:420]
            w1sb = wcat[0:10, 420:452]
            # fcat cols: caA [0:12), caB [12:24), b3 [24:25), hb [25:26),
            #            ones row [26:27) col-major? ones needs [1,128]:
            #            use fcat[0:1, 32:160]; accums [12, 160:416)
            fcat = wsb.tile([128, 416], dt.float32)
            caAsb = fcat[0:12, 0:12]
            caBsb = fcat[0:12, 12:24]
            b3sb = fcat[0:32, 24:25]
            hbsb = fcat[0:12, 25:26]
            onesb = fcat[0:1, 32:160]
            accums = fcat[0:12, 160:160 + NB * (R // G)]

            with tc.tile_pool(name="wstg", bufs=1) as wstg:
                wf = wstg.tile([128, 452], dt.float32)
                nc.vector.memset(wf[:], 0.0)
                for b in range(3):
                    nc.sync.dma_start(wf[0:97, b * 64:(b + 1) * 64], w2m_d[b])
                    nc.sync.dma_start(wf[0:128, 192 + b * 32:192 + (b + 1) * 32],
                                      w3a_d[b])
                    nc.sync.dma_start(wf[64:128, 288 + b * 32:288 + (b + 1) * 32],
                                      w3b_d[b])
                    nc.sync.dma_start(wf[0:96, 384 + b * 12:384 + (b + 1) * 12],
                                      hwm_d[b])
                nc.sync.dma_start(wf[0:10, 420:452], w1m_d[:])
                nc.vector.tensor_copy(wcat[:], wf[:])

            nc.vector.memset(fcat[:], 0.0)
            nc.sync.dma_start(caAsb, caA_d[:])
            nc.sync.dma_start(caBsb, caB_d[:])
            nc.sync.dma_start(b3sb, b3_d[:])
            nc.sync.dma_start(hbsb, hb_d[:])
            nc.vector.memset(onesb, 1.0)

            # ---- stage A0: x -> bf16 padded-row DRAM tensor ----
            def stage_a0():
                with tc.tile_pool(name="a0", bufs=1) as a0:
                    xt = a0.tile([128, 4, 512], dt.float32, name="xt")
                    xsb = a0.tile([128, 4, XW], dt.bfloat16, name="xsb")
                    zz = a0.tile([2, XW], dt.bfloat16, name="zz")
                    nc.sync.dma_start(
                        xt[:], xb[:, :].rearrange("(b p) w -> p b w", p=128))
                    nc.vector.memset(xsb[:], 0.0)
                    nc.vector.memset(zz[:], 0.0)
                    nc.vector.tensor_copy(xsb[:, :, 1:513], xt[:])
                    nc.sync.dma_start(
                        x_row[0:H, :].rearrange("(b p) w -> p b w", p=128),
                        xsb[:])
                    nc.sync.dma_start(x_row[H:H + 2, :], zz[:])

            # ============ phase A: fused band pipeline ============
            def stage_a():
                with (
                    tc.tile_pool(name="abuf", bufs=1) as abuf,
                    tc.tile_pool(name="ymid", bufs=1) as ymid,
                    tc.tile_pool(name="atmp", bufs=1) as atmp,
                    tc.tile_pool(name="aps", bufs=4, space="PSUM") as aps,
                ):
                    xrep = abuf.tile([10, 2 * 19, XW], dt.bfloat16,
                                     name="xrep")
                    b1r = [abuf.tile([97, CAP, XW], dt.bfloat16,
                                     name=f"b1r{i}") for i in range(2)]
                    b2r = [abuf.tile([128, CAP, XW], dt.bfloat16,
                                     name=f"b2r{i}") for i in range(2)]
                    b3r = [abuf.tile([96, CAP, XW], dt.bfloat16,
                                     name=f"b3r{i}") for i in range(2)]
                    # zero-init: unwritten slots read as zero padding
                    for t in b1r + b2r + b3r:
                        nc.vector.memset(t[:], 0.0)
                    nc.vector.memset(xrep[:], 0.0)
                    nc.vector.memset(xrep[0:1, :, :], 1.0)
                    for t in b1r:
                        nc.vector.memset(t[96:97, :, :], 1.0)

                    xr_handle = x_row[:, :]

                    def xgather(k, lo1, hi1):
                        """xrep[1+3a+b, uo+u, c] = x_row[lo1-1+u+a, c+b]"""
                        uo = (k % 2) * 19
                        for a in range(3):
                            s0 = max(lo1 - 1 + a, 0)   # rows >= H are zero pad
                            s1 = hi1 - 1 + a
                            u0 = uo + s0 - (lo1 - 1 + a)
                            cnt = s1 - s0
                            if cnt <= 0:
                                continue
                            src = bass.AP(
                                xr_handle.tensor,
                                xr_handle.offset + s0 * XW,
                                [[1, 3], [XW, cnt], [1, 514]])
                            if "g" not in skip:
                                nc.sync.dma_start(
                                    xrep[1 + 3 * a:4 + 3 * a,
                                         u0:u0 + cnt, 0:514], src)

                    def prelu_dve(dst, ps, alpha, g, p):
                        if "p" in skip:
                            return
                        tmp = atmp.tile([64, G, 512], dt.float32, tag="ptmp",
                                        name="ptmp")
                        nc.vector.tensor_scalar_mul(
                            tmp[0:p, 0:g, :], ps[0:p, 0:g, :], alpha)
                        nc.vector.tensor_max(dst, tmp[0:p, 0:g, :],
                                             ps[0:p, 0:g, :])

                    def conv1(k, buf):
                        r0 = R * k
                        lo1 = r0 + 3 if k else 0
                        hi1 = min(r0 + R + 3, H)
                        xgather(k, lo1, hi1)
                        q = lo1
                        while q < hi1:
                            g = min(G, hi1 - q)
                            ps = aps.tile([64, G, 512], dt.float32, tag="ps",
                                          name="psT")
                            for j in range(g):
                                u = (k % 2) * 19 + (q + j) - lo1
                                nc.tensor.matmul(ps[0:32, j, :], w1sb,
                                                 xrep[0:10, u, 0:512],
                                                 start=True, stop=True)
                            s0 = (q - r0) + 3
                            dst = buf[0:32, s0:s0 + g, 1:513]
                            prelu_dve(dst, ps, alpha1, g, 32)
                            q += g
                        s_hi = (hi1 - r0) + 3
                        if s_hi < CAP:
                            nc.vector.memset(buf[0:32, s_hi:CAP, :], 0.0)
                        # replicas: g1 <- g0(+1), g2 <- g0(+2)
                        c1 = (lo1 - r0) + 2
                        if "c" not in skip:
                            nc.scalar.dma_start(buf[32:64, c1:13, :],
                                                buf[0:32, c1 + 1:14, :])
                            nc.scalar.dma_start(buf[64:96, c1 - 1:12, :],
                                                buf[0:32, c1 + 1:14, :])
                            nc.scalar.dma_start(buf[32:64, 13:21, :],
                                                buf[0:32, 14:22, :])
                            nc.scalar.dma_start(buf[64:96, 12:20, :],
                                                buf[0:32, 14:22, :])

                    def conv2(k, src, buf):
                        r0 = R * k
                        lo2 = r0 + 2 if k else 0
                        hi2 = min(r0 + R + 2, H)
                        q = lo2
                        while q < hi2:
                            g = min(G, hi2 - q)
                            ps = aps.tile([64, G, 512], dt.float32, tag="ps",
                                          name="psT")
                            for j in range(g):
                                t1 = (q + j) - r0 + 2
                                for b in range(3):
                                    nc.tensor.matmul(
                                        ps[:, j, :],
                                        w2sb[:, b * 64:(b + 1) * 64],
                                        src[0:97, t1, b:b + 512],
                                        start=(b == 0), stop=(b == 2))
                            s0 = (q - r0) + 4
                            dst = buf[0:64, s0:s0 + g, 1:513]
                            if "p" not in skip:
                                nc.scalar.activation(dst, ps[0:64, 0:g, :],
                                                     AFT.Prelu,
                                                     bias=fcat[0:64, 26:27],
                                                     scale=1.0, alpha=alpha2)
                            q += g
                        s_hi = (hi2 - r0) + 4
                        if s_hi < CAP:
                            nc.vector.memset(buf[0:64, s_hi:CAP, :], 0.0)
                        c2 = (lo2 - r0) + 3
                        if "c" not in skip:
                            nc.gpsimd.dma_start(buf[64:128, c2:13, :],
                                                buf[0:64, c2 + 1:14, :])
                            nc.gpsimd.dma_start(buf[64:128, 13:21, :],
                                                buf[0:64, 14:22, :])

                    def conv3(k, src, buf):
                        r0 = R * k
                        lo3 = r0 + 1 if k else 0
                        hi3 = min(r0 + R + 1, H)
                        q = lo3
                        while q < hi3:
                            g = min(G, hi3 - q)
                            ps = aps.tile([64, G, 512], dt.float32, tag="ps",
                                          name="psT")
                            for j in range(g):
                                t2 = (q + j) - r0 + 3
                                for b in range(3):
                                    nc.tensor.matmul(
                                        ps[0:32, j, :],
                                        w3asb[:, b * 32:(b + 1) * 32],
                                        src[0:128, t2, b:b + 512],
                                        start=(b == 0), stop=False)
                                for b in range(3):
                                    nc.tensor.matmul(
                                        ps[0:32, j, :],
                                        w3bsb[:, b * 32:(b + 1) * 32],
                                        src[64:128, t2 + 1, b:b + 512],
                                        start=False, stop=(b == 2))
                            s0 = (q - r0) + 5
                            if "p" not in skip:
                                nc.scalar.activation(
                                    buf[0:32, s0:s0 + g, 1:513],
                                    ps[0:32, 0:g, :], AFT.Prelu,
                                    bias=b3sb, scale=1.0, alpha=alpha3)
                            q += g
                        s_hi = (hi3 - r0) + 5
                        if s_hi < CAP:
                            nc.vector.memset(buf[0:32, s_hi:CAP, :], 0.0)
                        c3 = (lo3 - r0) + 4
                        if "c" not in skip:
                            nc.gpsimd.dma_start(buf[32:64, c3:13, :],
                                                buf[0:32, c3 + 1:14, :])
                            nc.gpsimd.dma_start(buf[64:96, c3 - 1:12, :],
                                                buf[0:32, c3 + 1:14, :])
                            nc.gpsimd.dma_start(buf[32:64, 13:21, :],
                                                buf[0:32, 14:22, :])
                            nc.gpsimd.dma_start(buf[64:96, 12:20, :],
                                                buf[0:32, 14:22, :])

                    def heads(k, src):
                        r0 = R * k
                        ystg = ymid.tile([12, R, 512], dt.bfloat16,
                                         tag="ystg", name="ystg")
                        for jj in range(R // G):
                            ps = aps.tile([64, G, 512], dt.float32, tag="ps",
                                          name="psT")
                            for j in range(G):
                                th = jj * G + j + 4
                                for b in range(3):
                                    nc.tensor.matmul(
                                        ps[0:12, j, :],
                                        hwsb[:, b * 12:(b + 1) * 12],
                                        src[0:96, th, b:b + 512],
                                        start=(b == 0), stop=(b == 2))
                            idx = k * (R // G) + jj
                            if "p" not in skip:
                                nc.scalar.activation(
                                    ystg[:, jj * G:(jj + 1) * G, :],
                                    ps[0:12, :, :], AFT.Identity, bias=hbsb,
                                    scale=1.0,
                                    accum_out=accums[:, idx:idx + 1])
                        nc.sync.dma_start(y_dram[:, r0:r0 + R, :], ystg[:])

                    for k in range(NB):
                        cur1, cur2, cur3 = b1r[k % 2], b2r[k % 2], b3r[k % 2]
                        if k > 0 and "c" not in skip:
                            prv1, prv2, prv3 = (b1r[1 - k % 2], b2r[1 - k % 2],
                                                b3r[1 - k % 2])
                            nc.scalar.dma_start(cur1[0:96, 4:6, :],
                                                prv1[0:96, 20:22, :])
                            nc.gpsimd.dma_start(cur2[0:128, 4:6, :],
                                                prv2[0:128, 20:22, :])
                            nc.gpsimd.dma_start(cur3[0:96, 4:6, :],
                                                prv3[0:96, 20:22, :])
                        conv1(k, cur1)
                        conv2(k, cur1, cur2)
                        conv3(k, cur2, cur3)
                        heads(k, cur3)

            # ============ phase B: CA + softmax + blurs ============
            def stage_b():
                with (
                    tc.tile_pool(name="bsm", bufs=1) as bsm,
                    tc.tile_pool(name="bps1", bufs=1, space="PSUM") as bps1,
                    tc.tile_pool(name="bps", bufs=2, space="PSUM") as bps,
                    tc.tile_pool(name="bbl", bufs=1) as bbl,
                ):
                    bandf = bsm.tile([128, 36 * 128], dt.float32, name="bandf")
                    bandsb = bsm.tile([128, 36 * 128], dt.float32r,
                                      name="bandsb")
                    nc.sync.dma_start(bandf[:], bandP_d[:])
                    nc.vector.tensor_copy(bandsb[:], bandf[:])

                    # CA gating
                    total = bsm.tile([12, 1], dt.float32, name="total")
                    nc.vector.reduce_sum(total[:], accums,
                                         axis=mybir.AxisListType.X)
                    psA = bps1.tile([12, 1], dt.float32, tag="caps", name="psA")
                    nc.tensor.matmul(psA[:], caAsb, total[:],
                                     start=True, stop=True)
                    trelu = bsm.tile([12, 1], dt.float32, name="trelu")
                    nc.scalar.activation(trelu[:], psA[:], AFT.Relu)
                    psB = bps1.tile([12, 1], dt.float32, tag="caps", name="psB")
                    nc.tensor.matmul(psB[:], caBsb, trelu[:],
                                     start=True, stop=True)
                    g_gate = bsm.tile([12, 1], dt.float32, name="g_gate")
                    nc.scalar.activation(g_gate[:], psB[:], AFT.Sigmoid)
                    if debug:
                        nc.sync.dma_start(dbg["d_g"][:], g_gate[:])
                        nc.sync.dma_start(dbg["d_y"][:], y_dram[:])
                    g_row = bsm.tile([1, 12], dt.float32, name="g_row")
                    nc.sync.dma_start(g_row[:], g_gate[:])
                    psG = bps1.tile([128, 12], dt.float32, tag="gbc", name="psG")
                    nc.tensor.matmul(psG[:], onesb, g_row[:],
                                     start=True, stop=True)
                    gbc = bsm.tile([128, 12], dt.float32, name="gbc")
                    nc.vector.tensor_copy(gbc[:], psG[:])

                    # blur planes
                    FW = 4 * BS  # 2240
                    u = bbl.tile([128, FW], dt.float32r, name="u")
                    S2 = bbl.tile([128, FW], dt.float32r, name="S2")
                    S4 = bbl.tile([128, FW], dt.float32r, name="S4")
                    S8 = bbl.tile([128, FW], dt.float32r, name="S8")
                    S16 = bbl.tile([128, FW], dt.float32r, name="S16")
                    S5 = bbl.tile([128, FW], dt.float32r, name="S5")
                    S15 = bbl.tile([128, FW], dt.float32r, name="S15")
                    S25 = bbl.tile([128, FW], dt.float32r, name="S25")
                    unext = bbl.tile([128, FW], dt.float32r, name="unext")
                    t1 = bbl.tile([128, 512], dt.float32, name="t1")
                    t2 = bbl.tile([128, 512], dt.float32, name="t2")
                    ostg = bbl.tile([128, 4, 512], dt.float32, name="ostg")
                    nc.vector.memset(u[:].bitcast(dt.float32), 0.0)
                    nc.vector.memset(unext[:].bitcast(dt.float32), 0.0)

                    xt2 = bsm.tile([128, 4, 512], dt.float32, name="xt2")
                    nc.sync.dma_start(
                        xt2[:], xb[:, :].rearrange("(b p) w -> p b w", p=128))
                    uview = u[:].rearrange("p (b w) -> p b w", b=4)
                    nc.vector.tensor_copy(uview[:, :, DOFF:DOFF + 512], xt2[:])

                    yt = bsm.tile([128, 4, 4, 512], dt.bfloat16, name="yt")
                    ep = [bsm.tile([128, 4, 512], dt.float32, tag=f"exp{c}",
                                   name=f"ep{c}")
                          for c in range(4)]
                    tsum = bsm.tile([128, 4, 512], dt.float32, name="tsum")

                    cs = {5: 2, 15: 7, 25: 12}
                    ucur, unxt = u, unext
                    for stage in range(3):
                        # softmax for this head (channels 4*stage .. +4)
                        nc.sync.dma_start(
                            yt[:],
                            y_dram[4 * stage:4 * stage + 4, :, :].rearrange(
                                "c (b p) w -> p c b w", p=128))
                        for c in range(4):
                            cg = 4 * stage + c
                            nc.scalar.activation(ep[c][:], yt[:, c, :, :],
                                                 AFT.Exp,
                                                 scale=gbc[:, cg:cg + 1])
                        nc.vector.tensor_add(tsum[:], ep[0][:], ep[1][:])
                        nc.vector.tensor_add(tsum[:], tsum[:], ep[2][:])
                        nc.vector.tensor_add(tsum[:], tsum[:], ep[3][:])
                        nc.vector.reciprocal(tsum[:], tsum[:])
                        for c in range(4):
                            nc.vector.tensor_mul(ep[c][:], ep[c][:], tsum[:])

                        # shift-tree along W (horizontal box sums)
                        wv = FW - 24
                        nc.vector.tensor_add(S2[:, 0:wv], ucur[:, 0:wv],
                                             ucur[:, 1:1 + wv])
                        nc.vector.tensor_add(S4[:, 0:wv], S2[:, 0:wv],
                                             S2[:, 2:2 + wv])
                        nc.vector.tensor_add(S8[:, 0:wv], S4[:, 0:wv],
                                             S4[:, 4:4 + wv])
                        nc.vector.tensor_add(S16[:, 0:wv], S8[:, 0:wv],
                                             S8[:, 8:8 + wv])
                        nc.vector.tensor_add(S5[:, 0:wv], S4[:, 0:wv],
                                             ucur[:, 4:4 + wv])
                        nc.vector.tensor_sub(S15[:, 0:wv], S16[:, 0:wv],
                                             ucur[:, 15:15 + wv])
                        nc.vector.tensor_add(S25[:, 0:wv], S16[:, 0:wv],
                                             S8[:, 16:16 + wv])
                        nc.vector.tensor_add(S25[:, 0:wv], S25[:, 0:wv],
                                             ucur[:, 24:24 + wv])

                        Sk = {5: S5, 15: S15, 25: S25}
                        for t in range(4):
                            pk = {}
                            for kidx, kk in enumerate((5, 15, 25)):
                                ps = bps.tile([128, 512], dt.float32,
                                              tag=f"blur{kidx}",
                                              name=f"blur{kidx}")
                                rels = [r for r in (-1, 0, 1)
                                        if 0 <= t + r <= 3]
                                for ri, rel in enumerate(rels):
                                    idx = kidx * 12 + t * 3 + (rel + 1)
                                    off = (t + rel) * BS + DOFF - cs[kk]
                                    nc.tensor.matmul(
                                        ps[:],
                                        bandsb[:, idx * 128:(idx + 1) * 128],
                                        Sk[kk][:, off:off + 512],
                                        start=(ri == 0),
                                        stop=(ri == len(rels) - 1))
                                pk[kk] = ps
                            ub = ucur[:, t * BS + DOFF:t * BS + DOFF + 512]
                            nc.vector.tensor_mul(t1[:], ep[0][:, t, :], ub)
                            nc.vector.tensor_mul(t2[:], ep[1][:, t, :],
                                                 pk[5][:])
                            nc.vector.tensor_add(t1[:], t1[:], t2[:])
                            nc.vector.tensor_mul(t2[:], ep[2][:, t, :],
                                                 pk[15][:])
                            nc.vector.tensor_add(t1[:], t1[:], t2[:])
                            nc.vector.tensor_mul(t2[:], ep[3][:, t, :],
                                                 pk[25][:])
                            if stage < 2:
                                nc.vector.tensor_add(
                                    unxt[:, t * BS + DOFF:t * BS + DOFF + 512],
                                    t1[:], t2[:])
                            else:
                                nc.vector.tensor_add(ostg[:, t, :], t1[:],
                                                     t2[:])
                        if stage < 2:
                            ucur, unxt = unxt, ucur

                    nc.sync.dma_start(
                        outb[:, :].rearrange("(b p) w -> p b w", p=128),
                        ostg[:])

            def phases():
                if "A" in stages:
                    stage_a0()
                    stage_a()
                if "B" in stages:
                    stage_b()

            if loop_reps:
                with tc.For_i(0, loop_reps, 1):
                    phases()
            else:
                phases()

    nc.compile()
    return nc


class _Runner:
    """Cached PJRT runner: jit/NEFF compile once, execute many times."""

    def __init__(self, nc):
        import jax
        import concourse.mybir as mybir
        from concourse import bass2jax
        from jax.sharding import Mesh, PartitionSpec
        from jax.experimental.shard_map import shard_map

        bass2jax.install_neuronx_cc_hook()
        self.nc = nc
        in_names, out_names, out_avals, zero_outs = [], [], [], []
        partition_name = (nc.partition_id_tensor.name
                          if nc.partition_id_tensor else None)
        for alloc in nc.m.functions[0].allocations:
            if not isinstance(alloc, mybir.MemoryLocationSet):
                continue
            name = alloc.memorylocations[0].name
            if alloc.kind == "ExternalInput":
                if name != partition_name:
                    in_names.append(name)
            elif alloc.kind == "ExternalOutput":
                out_names.append(name)
                shape = tuple(alloc.tensor_shape)
                dtype = mybir.dt.np(alloc.dtype)
                out_avals.append(jax.core.ShapedArray(shape, dtype))
                zero_outs.append(np.zeros(shape, dtype))
        self.in_names = list(in_names)
        self.out_names = out_names
        self.out_avals = out_avals
        self.zero_outs = zero_outs
        n_params = len(in_names)
        n_outs = len(out_names)
        all_names = in_names + out_names
        if partition_name is not None:
            all_names.append(partition_name)

        def _body(*args):
            operands = list(args)
            if partition_name is not None:
                operands.append(bass2jax.partition_id_tensor())
            outs = bass2jax._bass_exec_p.bind(
                *operands,
                out_avals=tuple(out_avals),
                in_names=tuple(all_names),
                out_names=tuple(out_names),
                lowering_input_output_aliases=(),
                sim_require_finite=True,
                sim_require_nnan=True,
                nc=nc,
            )
            return tuple(outs)

        devices = jax.devices()[:NCORES]
        mesh = Mesh(np.asarray(devices), ("core",))
        in_specs = (PartitionSpec("core"),) * (n_params + n_outs)
        out_specs = (PartitionSpec("core"),) * n_outs
        self.sharded = jax.jit(
            shard_map(_body, mesh=mesh, in_specs=in_specs, out_specs=out_specs,
                      check_rep=False),
            keep_unused=True,
        )

    def concat_inputs(self, in_maps):
        return [
            np.concatenate([np.asarray(in_maps[c][nm]) for c in range(NCORES)],
                           axis=0)
            for nm in self.in_names
        ]

    def concat_zeros(self):
        return [np.zeros((NCORES * z.shape[0], *z.shape[1:]), z.dtype)
                for z in self.zero_outs]

    def __call__(self, in_maps):
        out_arrs = self.sharded(*self.concat_inputs(in_maps),
                                *self.concat_zeros())
        return [
            {nm: np.asarray(out_arrs[i]).reshape(NCORES,
                                                 *self.out_avals[i].shape)[c]
             for i, nm in enumerate(self.out_names)}
            for c in range(NCORES)
        ]


def _get_runner(alpha1, alpha2, alpha3, loop_reps=0, stages=None):
    if stages is None:
        stages = STAGES
    key = ("runner", alpha1, alpha2, alpha3, DEBUG, loop_reps, stages, SKIP)
    if key not in _CACHE:
        key_nc = (alpha1, alpha2, alpha3, DEBUG, loop_reps, stages, SKIP)
        if key_nc not in _CACHE:
            _CACHE[key_nc] = _build(alpha1, alpha2, alpha3, debug=DEBUG,
                                    loop_reps=loop_reps, stages=stages)
        _CACHE[key] = _Runner(_CACHE[key_nc])
    return _CACHE[key]


def make_in_maps(inputs):
    x = np.asarray(inputs["x"], np.float32)   # [8,1,512,512]
    packed = _pack_host(inputs)
    in_maps = []
    for i in range(NCORES):
        m = {"xb": np.ascontiguousarray(x[i, 0])}
        m.update({k: packed[k] for k in ("w1m", "w2m", "w3a", "w3b", "hwm",
                                         "caA", "caB", "bandP", "b3", "hb")})
        in_maps.append(m)
    return in_maps


def kernel(**inputs):
    runner = _get_runner(float(inputs["a1"]), float(inputs["a2"]),
                         float(inputs["a3"]))
    results = runner(make_in_maps(inputs))
    out = np.stack([results[i]["outb"] for i in range(NCORES)])
    globals()["_LAST_RESULTS"] = results
    return out.reshape(8, 1, H, W).astype(np.float32)
